# revision 2
# baseline (speedup 1.0000x reference)
"""MoE BERT block kernel for 8 Trainium2 NeuronCores.

Strategy: tensor-parallel over the expert FFN's INTER dimension. The router
(gate matmul + softmax + top-2) is a ~134 MFLOP computation done on the host
in float64 while packing the inputs; tokens are gathered into per-expert
segments on the host. Every core receives ALL 16384 token-expert pairs but
only a 512-wide slice of each expert's inter dimension:

    core c:  h_c   = gelu(Wup[e][c*512:(c+1)*512] @ x + bup_slice)   per token
             y_c   = Wdown[e][:, c*512:(c+1)*512] @ h_c              (partial)

gelu is elementwise in the inter dim, so y = sum_c y_c exactly. The host sums
the 8 f16 partials (float32 accumulate), adds bdown, and scatter-adds w * y.

Why this beats expert-parallel: per-core work is 64 matmul columns x 16384
tokens = 1.048M PE cycles regardless of the router outcome — perfect load
balance with zero token padding (expert-parallel pays for the heaviest
expert's 2161 tokens = 1.107M cycles). SBUF weight footprint is unchanged
(1/8 of every expert = 16.8 MB bf16).

Token tiles are sized from the actual per-expert loads (first tile of each
expert 512, remainder spread evenly, all >= 128 so every matmul's streaming
time stays above the ~53ns LDWEIGHTS floor). The Bass program is built per
load-signature and cached, so any router outcome is handled correctly.
"""

import os

os.environ.setdefault("MYCRO_LOCAL_CACHE", "1")

import numpy as np
import ml_dtypes

import concourse.bass as bass
import concourse.bacc as bacc
import concourse.mybir as mybir
import concourse.tile as tile
from concourse.bass_utils import run_bass_kernel_spmd

NUM_EXPERTS = 8
TOP_K = 2
H = 1024
I = 4096
P = 128
NCORES = 8
SLICE = I // NCORES  # 512 inter rows per core
KO = H // P  # 8 contraction tiles for the up matmul
IOL = SLICE // P  # 4 local inter tiles (psum partitions up / contraction down)
HO = H // P  # 8 output tiles for the down matmul
NMAX = 512  # max token tile (psum bank holds 512 f32)

BF16 = mybir.dt.bfloat16
F16 = mybir.dt.float16
F32 = mybir.dt.float32

_programs = {}  # schedule tuple -> compiled Bacc
last_results = None  # BassKernelResults of the most recent run (for profiling)


def _token_tiles(n):
    """Split n tokens into tiles: first tile NMAX, rest spread evenly in
    (128, 512]. The big first tile gives the DMA stream time to fill the
    pipeline before the next expert's weights are needed."""
    if n == 0:
        return []
    if n <= NMAX:
        return [n]
    k = -(-n // NMAX)  # ceil
    if n - NMAX <= (k - 1) * NMAX:
        rest = n - NMAX
        kk = k - 1
        base, rem = divmod(rest, kk)
        return [NMAX] + [base + 1] * rem + [base] * (kk - rem)
    base, rem = divmod(n, k)
    return [base + 1] * rem + [base] * (k - rem)


def _build_program(schedule):
    """schedule: tuple of (expert, ntok) tiles, concatenated token order."""
    TT = sum(nt for _, nt in schedule)
    nc = bacc.Bacc("TRN2", target_bir_lowering=False)

    xt = nc.dram_tensor("xt", [P, KO * TT], BF16, kind="ExternalInput")
    wup = nc.dram_tensor("wup", [P, NUM_EXPERTS * KO * SLICE], BF16, kind="ExternalInput")
    wdn = nc.dram_tensor("wdn", [P, NUM_EXPERTS * IOL * H], BF16, kind="ExternalInput")
    bup = nc.dram_tensor("bup", [P, NUM_EXPERTS * IOL], F32, kind="ExternalInput")
    yt = nc.dram_tensor("yt", [H, TT], F16, kind="ExternalOutput")

    experts_in_order = []
    for e, _ in schedule:
        if not experts_in_order or experts_in_order[-1] != e:
            experts_in_order.append(e)

    with tile.TileContext(nc) as tc:
        with (
            tc.tile_pool(name="weights", bufs=1) as wpool,
            tc.tile_pool(name="xin", bufs=3) as xpool,
            tc.tile_pool(name="hmid", bufs=2) as hpool,
            tc.tile_pool(name="yout", bufs=8) as ypool,
            tc.tile_pool(name="psum_up", bufs=4, space="PSUM") as pu,
            tc.tile_pool(name="psum_dn", bufs=4, space="PSUM") as pd,
        ):
            yt_r = yt.ap().rearrange("(ho p) t -> p ho t", p=P)
            xt_ap = xt.ap()
            wup_ap = wup.ap()
            wdn_ap = wdn.ap()

            wup_sb = wpool.tile([P, NUM_EXPERTS, KO, SLICE], BF16, tag="wup")
            wdn_sb = wpool.tile([P, NUM_EXPERTS, IOL, H], BF16, tag="wdn")
            bup_sb = wpool.tile([P, NUM_EXPERTS * IOL], F32, tag="bup")

            # Zeroed tile for warmup / keep-alive matmuls: no DMA dependency,
            # so the PE starts immediately and accumulates busy time toward
            # the 3us full-clock ramp while tile 0's operands stream in.
            # They add 0*0 = 0 into tile 0's first live psum group (exact).
            xw_sb = wpool.tile([P, NMAX], BF16, tag="warmx")
            nc.vector.memset(xw_sb[:], 0.0)

            def dma_wup(e, ko):
                col = (e * KO + ko) * SLICE
                nc.sync.dma_start(wup_sb[:, e, ko], wup_ap[:, col : col + SLICE])

            def dma_wdn(e, io):
                col = (e * IOL + io) * H
                nc.sync.dma_start(wdn_sb[:, e, io], wdn_ap[:, col : col + H])

            # --- Startup DMA order (sync ring executes in issue order):
            # tile 0's x + first expert's up weights interleaved per-ko so the
            # ko-major first tile can start after ~0.25MB; then the first
            # expert's down weights (needed ~9us in), then bup.
            e0, n0 = schedule[0]
            x0_sb = xpool.tile([P, KO, NMAX], BF16, tag="x")
            x0_r = xt_ap[:, 0 : KO * n0].rearrange("p (ko t) -> p ko t", ko=KO)
            for ko in range(KO):
                nc.sync.dma_start(x0_sb[:, ko, :n0], x0_r[:, ko])
                dma_wup(e0, ko)
            for io in range(IOL):
                dma_wdn(e0, io)
            nc.sync.dma_start(bup_sb[:], bup.ap())

            # Remaining experts' weights are issued interleaved with the tile
            # loop's x DMAs (3 chunks per tile) so the sync ring never builds
            # a bulk-weight backlog ahead of a latency-critical x tile.
            pending = []
            for e in experts_in_order[1:]:
                pending.extend(("up", e, ko) for ko in range(KO))
                pending.extend(("dn", e, io) for io in range(IOL))
            pending.reverse()  # pop from the end in order

            def issue_weight_chunks(k):
                for _ in range(min(k, len(pending))):
                    kind, e, i = pending.pop()
                    (dma_wup if kind == "up" else dma_wdn)(e, i)

            off = 0
            for t, (e, ntok) in enumerate(schedule):
                if t == 0:
                    x_sb = x0_sb
                else:
                    x_sb = xpool.tile([P, KO, NMAX], BF16, tag="x")
                    nc.sync.dma_start(
                        x_sb[:, :, :ntok],
                        xt_ap[:, KO * off : KO * (off + ntok)].rearrange(
                            "p (ko t) -> p ko t", ko=KO
                        ),
                    )
                    issue_weight_chunks(3)

                # --- Up-projection + exact (erf) GELU: h tile [512, ntok].
                h_sb = hpool.tile([P, IOL, NMAX], BF16, tag="h")
                if t == 0:
                    # ko-major across all 4 io psum groups: each ko step needs
                    # only x0[ko] + wup[e0,ko] (~0.25MB), matching the DMA
                    # arrival cadence. Warmup fillers bridge the cold start.
                    pss = [pu.tile([P, NMAX], F32, tag="pu", name=f"pu{j}") for j in range(IOL)]
                    nc.tensor.matmul(
                        pss[0][:, :ntok], lhsT=xw_sb[:, :P], rhs=xw_sb[:, :ntok],
                        start=True, stop=False,
                    )
                    for _ in range(6):
                        nc.tensor.matmul(
                            pss[0][:, :ntok], lhsT=xw_sb[:, :P], rhs=xw_sb[:, :ntok],
                            start=False, stop=False,
                        )
                    for ko in range(KO):
                        for j in range(IOL):
                            nc.tensor.matmul(
                                pss[j][:, :ntok],
                                lhsT=wup_sb[:, e, ko, j * P : (j + 1) * P],
                                rhs=x_sb[:, ko, :ntok],
                                start=(ko == 0 and j != 0),
                                stop=(ko == KO - 1),
                            )
                        if ko < KO - 1:
                            # Keep-alive against DMA-arrival jitter.
                            nc.tensor.matmul(
                                pss[0][:, :ntok], lhsT=xw_sb[:, :P],
                                rhs=xw_sb[:, :ntok], start=False, stop=False,
                            )
                    for j in range(IOL):
                        nc.scalar.activation(
                            h_sb[:, j, :ntok], pss[j][:, :ntok],
                            mybir.ActivationFunctionType.Gelu,
                            bias=bup_sb[:, e * IOL + j : e * IOL + j + 1],
                            scale=1.0,
                        )
                else:
                    # io-major (contraction inner): each group's GELU drains
                    # while the next group accumulates — no end-of-tile burst.
                    for io in range(IOL):
                        ps = pu.tile([P, NMAX], F32, tag="pu", name=f"pu{io}")
                        for ko in range(KO):
                            nc.tensor.matmul(
                                ps[:, :ntok],
                                lhsT=wup_sb[:, e, ko, io * P : (io + 1) * P],
                                rhs=x_sb[:, ko, :ntok],
                                start=(ko == 0),
                                stop=(ko == KO - 1),
                            )
                        nc.scalar.activation(
                            h_sb[:, io, :ntok], ps[:, :ntok],
                            mybir.ActivationFunctionType.Gelu,
                            bias=bup_sb[:, e * IOL + io : e * IOL + io + 1],
                            scale=1.0,
                        )

                # --- Down-projection partial: y tile [1024, ntok] f16.
                # ho-major, contraction (4 io steps) inner; io order 0..3 so
                # ho0's first matmuls overlap the last GELU. The psum->f16
                # cast (DVE) and the y DMA (Activation-engine HWDGE ring, so
                # it never queues ahead of sync-ring x/weight inputs) overlap
                # the next ho's matmuls.
                for ho in range(HO):
                    ps = pd.tile([P, NMAX], F32, tag="pd", name=f"pd{ho % 4}")
                    for io in range(IOL):
                        nc.tensor.matmul(
                            ps[:, :ntok],
                            lhsT=wdn_sb[:, e, io, ho * P : (ho + 1) * P],
                            rhs=h_sb[:, io, :ntok],
                            start=(io == 0),
                            stop=(io == IOL - 1),
                        )
                    y_sb = ypool.tile([P, NMAX], F16, tag="y")
                    nc.vector.tensor_scalar_add(y_sb[:, :ntok], ps[:, :ntok], 0.0)
                    nc.scalar.dma_start(yt_r[:, ho, off : off + ntok], y_sb[:, :ntok])
                off += ntok

    nc.compile()
    return nc


def _get_program(schedule):
    key = tuple(schedule)
    if key not in _programs:
        _programs[key] = _build_program(key)
    return _programs[key]


def _route(X64, Wg64):
    """Replicates the reference router: softmax over gate logits, top-2."""
    T = X64.shape[0]
    logits = X64 @ Wg64.T  # [T, E]
    logits -= logits.max(axis=-1, keepdims=True)
    p = np.exp(logits)
    p /= p.sum(axis=-1, keepdims=True)
    i1 = np.argmax(p, axis=-1)
    rows = np.arange(T)
    w1 = p[rows, i1]
    p2 = p.copy()
    p2[rows, i1] = -1.0
    i2 = np.argmax(p2, axis=-1)
    w2 = p[rows, i2]
    return i1, w1, i2, w2


def kernel(hidden_states, Wg, Wup, bup, Wdown, bdown):
    global last_results
    hidden_states = np.asarray(hidden_states)
    orig_shape = hidden_states.shape
    X = np.ascontiguousarray(hidden_states, dtype=np.float32).reshape(-1, H)
    T = X.shape[0]
    Wg = np.asarray(Wg, dtype=np.float32)
    Wup = np.asarray(Wup, dtype=np.float32)
    bup = np.asarray(bup, dtype=np.float32)
    Wdown = np.asarray(Wdown, dtype=np.float32)
    bdown = np.asarray(bdown, dtype=np.float32)

    # --- Router on host (float64 for a faithful top-2 ordering) ---
    i1, w1, i2, w2 = _route(X.astype(np.float64), Wg.astype(np.float64))

    # --- Dispatch: gather tokens into per-expert segments (i1 then i2) ---
    seg_idx, seg_wts = [], []
    schedule = []
    for e in range(NUM_EXPERTS):
        sel1 = np.nonzero(i1 == e)[0]
        sel2 = np.nonzero(i2 == e)[0]
        idx = np.concatenate([sel1, sel2])
        wts = np.concatenate([w1[sel1], w2[sel2]])
        seg_idx.append(idx)
        seg_wts.append(wts)
        schedule.extend((e, nt) for nt in _token_tiles(idx.size))
    schedule = tuple(schedule)
    idx_all = np.concatenate(seg_idx)

    # --- Pack device inputs ---
    Xb = X.astype(ml_dtypes.bfloat16)
    Xsel = Xb[idx_all]  # [TT, H]
    blocks = []
    o = 0
    for _, nt in schedule:
        blk = Xsel[o : o + nt].T.reshape(KO, P, nt)  # [KO, P, nt]
        blocks.append(blk.transpose(1, 0, 2).reshape(P, -1))
        o += nt
    xt_dev = np.ascontiguousarray(np.concatenate(blocks, axis=1))

    Wup16 = Wup.astype(ml_dtypes.bfloat16)
    Wdn16 = Wdown.astype(ml_dtypes.bfloat16)
    in_maps = []
    for c in range(NCORES):
        rows = slice(c * SLICE, (c + 1) * SLICE)
        wup_c = np.empty((P, NUM_EXPERTS * KO * SLICE), dtype=ml_dtypes.bfloat16)
        wdn_c = np.empty((P, NUM_EXPERTS * IOL * H), dtype=ml_dtypes.bfloat16)
        bup_c = np.empty((P, NUM_EXPERTS * IOL), dtype=np.float32)
        for e in range(NUM_EXPERTS):
            # lhsT layout for up: [k partition, io rows] per ko chunk
            w = Wup16[e][rows, :].T.reshape(KO, P, SLICE).transpose(1, 0, 2)
            wup_c[:, e * KO * SLICE : (e + 1) * KO * SLICE] = w.reshape(P, -1)
            # lhsT layout for down: [local-inter partition, H cols] per io chunk
            d = Wdn16[e][:, rows].T.reshape(IOL, P, H).transpose(1, 0, 2)
            wdn_c[:, e * IOL * H : (e + 1) * IOL * H] = d.reshape(P, -1)
            bup_c[:, e * IOL : (e + 1) * IOL] = bup[e][rows].reshape(IOL, P).T
        in_maps.append(
            {
                "xt": xt_dev,
                "wup": np.ascontiguousarray(wup_c),
                "wdn": np.ascontiguousarray(wdn_c),
                "bup": bup_c,
            }
        )

    # --- Run the Bass kernel on all 8 cores ---
    nc = _get_program(schedule)
    last_results = run_bass_kernel_spmd(nc, in_maps, core_ids=list(range(NCORES)))

    # --- Combine: sum the 8 inter-slice partials, add bdown, scatter w * y ---
    ysum = np.zeros((H, sum(nt for _, nt in schedule)), dtype=np.float32)
    for c in range(NCORES):
        ysum += np.asarray(last_results.results[c]["yt"]).astype(np.float32)

    out = np.zeros((T, H), dtype=np.float32)
    o = 0
    for e in range(NUM_EXPERTS):
        n = seg_idx[e].size
        if n == 0:
            continue
        Y = ysum[:, o : o + n].T + bdown[e]
        out[seg_idx[e]] += seg_wts[e][:, None].astype(np.float32) * Y
        o += n
    return out.reshape(orig_shape)
